# revision 6
# baseline (speedup 1.0000x reference)
"""DiffusionNetBlock Bass/Tile kernel for 8 trn2 NeuronCores.

Sharding: 2 cores per batch sample (B=4); each core owns half the
vertices (rows). The spectral projection is computed over the own half
and summed across the pair with one 8-core AllReduce (4 sample slots).
Each core builds the FULL fp16 x_diffuse gather table locally
(replicated stage-2 work beats a 2-rank collective). Sparse gradient
SpMM: edges are bucketed by destination 128-row block (sorted on host),
padded to CAP chunks of 128 edges; cols are gathered from the local
table via one indirect DMA PER 128-EDGE CHUNK ([128,1] offsets -> one
row per partition; TRN2 HW consumes exactly one index per partition,
unlike CoreSim); the segment sum is a one-hot
matmul per chunk (lhsT = gathered g, rhs = [eq*vx | eq*vy]) accumulating
gx^T|gy^T in PSUM. Rotation, tanh, MLP and residual are fused per
256-row superblock in [feature, vertex] layout; the host transposes the
output back. evecs^T / x_in^T come pre-transposed (fp16) from the host.
"""
import sys

for _p in ("/opt/trn_rl_repo", "/opt/pypackages"):
    if _p not in sys.path:
        sys.path.append(_p)

from contextlib import ExitStack

import numpy as np

import bass_rust
import concourse.bass as bass
import concourse.mybir as mybir
import concourse.tile as tile
from concourse.bass_utils import run_bass_kernel_spmd
from concourse.masks import make_identity
from concourse.vector_clock import ScopedClock

F32 = mybir.dt.float32
F16 = mybir.dt.float16
I32 = mybir.dt.int32
I16 = mybir.dt.int16
AF = mybir.ActivationFunctionType
ALU = mybir.AluOpType

# Problem dims (hardcoded per spec)
B, V, C, K, E, H = 4, 50000, 128, 128, 400000, 128
N_CORES = 8
VH = V // 2            # 25000 vertices per core
NBLK = (VH + 127) // 128   # 196 blocks of 128 rows
VHP = NBLK * 128       # 25088 padded
NSUP = NBLK // 2       # 98 superblocks of 256 rows
CAP_DEFAULT = 10       # 128-edge chunks per block (mean 8, 8-sigma margin)


# ---------------------------------------------------------------------------
# Tile drain patch: walrus in this container rejects CTRL (Drain)
# instructions carrying sem waits. Move the kernel-tail drain's waits onto
# single-wait NOPs (sequential issue on SP makes the chain equivalent).
# ---------------------------------------------------------------------------
_MAX_WAIT_NOPS = 48


def _drain_and_barrier_split(self, tick_clock, wait_clock):
    nc = self.nc
    nops = [
        nc.sync.nop(nofuse=True, hint=f"drain_wait_split_{i}")
        for i in range(_MAX_WAIT_NOPS)
    ]
    drain_inst = nc.sync.drain().ins
    wait_clock.add_sem_waits(drain_inst, ScopedClock({None: tick_clock.global_clock}))
    si = drain_inst.sync_info
    waits = list(si.on_wait) if si is not None else []
    assert len(waits) <= _MAX_WAIT_NOPS, f"too many drain waits: {len(waits)}"
    if waits:
        drain_inst.sync_info = bass_rust.SyncInfo(
            on_wait=[], on_update=list(si.on_update)
        )
        for w, nop in zip(waits, nops):
            nop.ins.sync_info = bass_rust.SyncInfo(on_wait=[w], on_update=[])

    nc.all_engine_barrier()
    assert self.sems is not None
    popped = nc._tile_sem_poison_stack.pop()
    assert popped is self._sem_poison
    nc.clear_and_free_semaphores(list(self.sems.allocated().values()))
    nc.all_engine_barrier()


tile.TileContext._drain_and_barrier = _drain_and_barrier_split

_spill_id = [0]


def _spill_waits(nc, cap=1, drain_cap=1):
    """Walrus in this container bounds sem-waits per instruction (CTRL ~1,
    compute/DMA ~2). Move excess waits onto same-engine NOPs inserted right
    before the instruction (engines issue in order, so a chain of
    single-wait NOPs is equivalent to one multi-wait instruction)."""
    for f in nc.m.functions:
        for bb in f.blocks:
            out = []
            changed = False
            for inst in bb.instructions:
                si = inst.sync_info
                waits = list(si.on_wait) if si is not None else []
                c = drain_cap if isinstance(inst, mybir.InstDrain) else cap
                if len(waits) > c:
                    keep = waits[len(waits) - c:] if c else []
                    for w in waits[:len(waits) - c]:
                        nop = mybir.InstNoOp(
                            name=f"wspill-{_spill_id[0]}", ins=[], outs=[])
                        _spill_id[0] += 1
                        nop.engine = inst.engine
                        nop.sync_info = bass_rust.SyncInfo(
                            on_wait=[w], on_update=[])
                        nc.register_instruction(nop, overwrite=True)
                        out.append(nop)
                    inst.sync_info = bass_rust.SyncInfo(
                        on_wait=keep, on_update=list(si.on_update))
                    changed = True
                out.append(inst)
            if changed:
                bb.instructions = out


# ---------------------------------------------------------------------------
# Program builder
# ---------------------------------------------------------------------------
def build_nc(nblk=NBLK, cap=CAP_DEFAULT, n_cores=N_CORES, debug_taps=False):
    """Build the SPMD Bass program. All per-core variation is in the data."""
    vhp = nblk * 128
    nsup = nblk // 2
    assert nblk % 4 == 0
    tvp = 2 * vhp  # gather table rows
    nq = nblk // 4  # quad count for batched streaming

    nc = bass.Bass("TRN2", target_bir_lowering=False, debug=False,
                   num_devices=n_cores)

    def din(name, shape, dt):
        return nc.dram_tensor(name, shape, dt, kind="ExternalInput").ap()

    x_in = din("x_in", [vhp, C], F16)        # stage-1 stream
    x_inT = din("x_inT", [C, vhp], F16)      # MLP rhs + residual
    mass = din("mass", [vhp, 1], F32)
    ev_own = din("ev_own", [vhp, K], F16)    # stage-1 lhsT stream
    evT_own = din("evT_own", [K, vhp], F16)  # resident, stage-2 own
    evoT = din("evoT", [K, vhp], F16)        # stage-2 other-half stream
    decay = din("decay", [K, C], F32)
    a_re = din("a_re", [C, C], F16)
    a_imn = din("a_imn", [C, C], F16)   # -A_im
    w0a = din("w0a", [C, H], F16)
    w0b = din("w0b", [C, H], F16)
    w0c = din("w0c", [C, H], F16)
    w1 = din("w1", [H, C], F16)
    b0 = din("b0", [H, 1], F32)
    b1 = din("b1", [C, 1], F32)
    cols = din("cols", [nblk, 128, cap], I32)
    # rvxy packs rows | vx | vy along the last axis
    rvxy = din("rvxy", [nblk, 128, 3 * cap], F16)
    # sel[s] = 1.0 iff this core's sample is slot s (same program, data-driven)
    sel = din("sel", [128, 4], F32)

    outT = nc.dram_tensor("outT", [C, vhp], F32, kind="ExternalOutput").ap()
    if debug_taps:
        dbg_xs = nc.dram_tensor("dbg_xs", [K, C], F32, kind="ExternalOutput").ap()
        dbg_tab = nc.dram_tensor("dbg_tab", [1024, C], F16, kind="ExternalOutput").ap()
        dbg_r1 = nc.dram_tensor("dbg_r1", [C, 512], F16, kind="ExternalOutput").ap()
        dbg_xgt = nc.dram_tensor("dbg_xgt", [C, 256], F16, kind="ExternalOutput").ap()
        dbg_hr = nc.dram_tensor("dbg_hr", [H, 256], F16, kind="ExternalOutput").ap()
        dbg_g = nc.dram_tensor("dbg_g", [128, 2 * cap * C], F16, kind="ExternalOutput").ap()
        dbg_sp = nc.dram_tensor("dbg_sp", [128, cap * 256], F16, kind="ExternalOutput").ap()

    nslot = 4
    xs_loc = nc.dram_tensor("xs_loc", [nslot * K, C], F32).ap()
    xs_sh = nc.dram_tensor("xs_sh", [nslot * K, C], F32,
                           addr_space="Shared").ap()
    table = nc.dram_tensor("table", [tvp, C], F16).ap()

    groups = [list(range(n_cores))]

    with tile.TileContext(nc) as tc, ExitStack() as ctx:
        cpool = ctx.enter_context(tc.tile_pool(name="const", bufs=1))
        rpool = ctx.enter_context(tc.tile_pool(name="resid", bufs=1))

        # iota over r, twice side by side: [128, 2, 128] (for [Sx | Sy])
        iota_i = cpool.tile([128, 2, 128], I16, tag="iota_i")
        nc.gpsimd.iota(iota_i[:], pattern=[[0, 2], [1, 128]], base=0,
                       channel_multiplier=0)
        iota2 = cpool.tile([128, 2, 128], F16, tag="iota2")
        nc.vector.tensor_copy(out=iota2[:], in_=iota_i[:])

        # constants
        decay_sb = cpool.tile([K, C], F32, tag="decay")
        nc.sync.dma_start(out=decay_sb[:], in_=decay[:, :])
        are_sb = cpool.tile([C, C], F16, tag="are")
        nc.sync.dma_start(out=are_sb[:], in_=a_re[:, :])
        aimn_sb = cpool.tile([C, C], F16, tag="aimn")
        nc.sync.dma_start(out=aimn_sb[:], in_=a_imn[:, :])
        w0a_sb = cpool.tile([C, H], F16, tag="w0a")
        nc.sync.dma_start(out=w0a_sb[:], in_=w0a[:, :])
        w0b_sb = cpool.tile([C, H], F16, tag="w0b")
        nc.sync.dma_start(out=w0b_sb[:], in_=w0b[:, :])
        w0c_sb = cpool.tile([C, H], F16, tag="w0c")
        nc.sync.dma_start(out=w0c_sb[:], in_=w0c[:, :])
        w1_sb = cpool.tile([H, C], F16, tag="w1")
        nc.sync.dma_start(out=w1_sb[:], in_=w1[:, :])
        b0_sb = cpool.tile([H, 1], F32, tag="b0")
        nc.sync.dma_start(out=b0_sb[:], in_=b0[:, :])
        b1_sb = cpool.tile([C, 1], F32, tag="b1")
        nc.sync.dma_start(out=b1_sb[:], in_=b1[:, :])
        sel_sb = cpool.tile([128, 4], F32, tag="sel")
        nc.sync.dma_start(out=sel_sb[:], in_=sel[:, :])
        mass_sb = cpool.tile([128, nblk], F32, tag="mass")
        nc.sync.dma_start(out=mass_sb[:],
                          in_=mass[:, :].rearrange("(t p) one -> p (t one)",
                                                   p=128))

        # residents
        evT = rpool.tile([K, vhp], F16, tag="evT")
        nc.sync.dma_start(out=evT[:], in_=evT_own[:, :])
        xdT = rpool.tile([C, vhp], F16, tag="xdT")

        # ---------------- stage 1: spectral projection ----------------
        with tc.tile_pool(name="s1", bufs=3) as s1p, \
             tc.tile_pool(name="ps_xs", bufs=1, space="PSUM") as ps_xs:
            psum_xs = ps_xs.tile([K, C], F32, tag="xs")
            for q in range(nq):
                qsl = slice(q * 512, (q + 1) * 512)
                xin_q = s1p.tile([128, 4, C], F16, tag="xin")
                ev_q = s1p.tile([128, 4, K], F16, tag="ev")
                nc.sync.dma_start(
                    out=xin_q[:],
                    in_=x_in[qsl, :].rearrange("(a p) c -> p a c", p=128))
                nc.sync.dma_start(
                    out=ev_q[:],
                    in_=ev_own[qsl, :].rearrange("(a p) c -> p a c", p=128))
                mx_q = s1p.tile([128, 4, C], F16, tag="mx")
                for a in range(4):
                    t = 4 * q + a
                    nc.vector.tensor_scalar_mul(mx_q[:, a, :], xin_q[:, a, :],
                                                mass_sb[:, t:t + 1])
                    nc.tensor.matmul(psum_xs[:], lhsT=ev_q[:, a, :],
                                     rhs=mx_q[:, a, :],
                                     start=(t == 0), stop=(t == nblk - 1))

            xs_sb = cpool.tile([K, C], F32, tag="xs_sb")
            nc.scalar.activation(out=xs_sb[:], in_=psum_xs[:], func=AF.Copy)
            for s in range(nslot):
                slot_sb = s1p.tile([K, C], F32, tag="slot")
                nc.vector.tensor_scalar_mul(slot_sb[:], xs_sb[:],
                                            sel_sb[:, s:s + 1])
                nc.sync.dma_start(out=xs_loc[s * K:(s + 1) * K, :],
                                  in_=slot_sb[:])

        nc.gpsimd.collective_compute(
            "AllReduce", ALU.add,
            ins=[xs_loc[:, :]], outs=[xs_sh[:, :]],
            replica_groups=groups,
        )

        # xs = decay * (own slot of the AllReduce result)
        slots_sb = cpool.tile([K, nslot * C], F32, tag="slots")
        for s in range(nslot):
            nc.sync.dma_start(out=slots_sb[:, s * C:(s + 1) * C],
                              in_=xs_sh[s * K:(s + 1) * K, :])
        xs_ar = cpool.tile([K, C], F32, tag="xs_ar")
        nc.vector.tensor_scalar_mul(xs_ar[:], slots_sb[:, 0:C],
                                    sel_sb[:, 0:1])
        for s in range(1, nslot):
            nc.vector.scalar_tensor_tensor(
                out=xs_ar[:], in0=slots_sb[:, s * C:(s + 1) * C],
                scalar=sel_sb[:, s:s + 1], in1=xs_ar[:],
                op0=ALU.mult, op1=ALU.add)
        xs32 = cpool.tile([K, C], F32, tag="xs32")
        nc.vector.tensor_tensor(out=xs32[:], in0=xs_ar[:], in1=decay_sb[:],
                                op=ALU.mult)
        xs16 = cpool.tile([K, C], F16, tag="xs16")
        nc.vector.tensor_copy(out=xs16[:], in_=xs32[:])
        if debug_taps:
            nc.sync.dma_start(out=dbg_xs[:, :], in_=xs32[:])

        # ---------------- stage 2: x_diffuse table + xdT ----------------
        with tc.tile_pool(name="s2", bufs=3) as s2p, \
             tc.tile_pool(name="ps_xd", bufs=3, space="PSUM") as ps_xd, \
             tc.tile_pool(name="ps_xt", bufs=3, space="PSUM") as ps_xt:
            for q in range(nq):
                td_q = s2p.tile([128, 4, C], F16, tag="td")
                for a in range(4):
                    t = 4 * q + a
                    vsl = slice(t * 128, (t + 1) * 128)
                    pxd = ps_xd.tile([128, C], F32, tag="xd")
                    nc.tensor.matmul(pxd[:], lhsT=evT[:, vsl], rhs=xs16[:],
                                     start=True, stop=True)
                    nc.scalar.activation(out=td_q[:, a, :], in_=pxd[:],
                                         func=AF.Copy)
                    pxt = ps_xt.tile([C, 128], F32, tag="xdt")
                    nc.tensor.matmul(pxt[:], lhsT=xs16[:], rhs=evT[:, vsl],
                                     start=True, stop=True)
                    nc.vector.tensor_copy(out=xdT[:, vsl], in_=pxt[:])
                nc.sync.dma_start(
                    out=table[q * 512:(q + 1) * 512, :].rearrange(
                        "(a p) c -> p a c", p=128),
                    in_=td_q[:])
                if debug_taps and q == 0:
                    nc.sync.dma_start(
                        out=dbg_tab[0:512, :].rearrange("(a p) c -> p a c", p=128),
                        in_=td_q[:])

            for q in range(nq):
                evo_q = s2p.tile([128, 4, 128], F16, tag="evo")
                nc.sync.dma_start(
                    out=evo_q[:],
                    in_=evoT[:, q * 512:(q + 1) * 512].rearrange(
                        "p (a c) -> p a c", a=4))
                td_q = s2p.tile([128, 4, C], F16, tag="td")
                for a in range(4):
                    pxd = ps_xd.tile([128, C], F32, tag="xd")
                    nc.tensor.matmul(pxd[:], lhsT=evo_q[:, a, :], rhs=xs16[:],
                                     start=True, stop=True)
                    nc.scalar.activation(out=td_q[:, a, :], in_=pxd[:],
                                         func=AF.Copy)
                nc.sync.dma_start(
                    out=table[vhp + q * 512:vhp + (q + 1) * 512, :].rearrange(
                        "(a p) c -> p a c", p=128),
                    in_=td_q[:])

        # ---------------- stages 3-5: SpMM + rotation + MLP ----------------
        with tc.tile_pool(name="s4", bufs=3) as s4p, \
             tc.tile_pool(name="s4b", bufs=3) as s4bp, \
             tc.tile_pool(name="ps_gxy", bufs=2, space="PSUM") as ps_gxy, \
             tc.tile_pool(name="ps_vb", bufs=2, space="PSUM") as ps_vb, \
             tc.tile_pool(name="ps_h", bufs=2, space="PSUM") as ps_h, \
             tc.tile_pool(name="ps_o", bufs=2, space="PSUM") as ps_o:
            for sp2 in range(nsup // 2):
                # batched loads + one gather for 2 superblocks (4 blocks)
                cols_t = s4p.tile([128, 4, cap], I32, tag="cols")
                nc.sync.dma_start(
                    out=cols_t[:],
                    in_=cols[4 * sp2:4 * sp2 + 4].rearrange("a p j -> p a j"))
                meta_t = s4p.tile([128, 4, 3 * cap], F16, tag="meta")
                nc.sync.dma_start(
                    out=meta_t[:],
                    in_=rvxy[4 * sp2:4 * sp2 + 4].rearrange("a p j -> p a j"))
                # one indirect DMA per 128-edge chunk: the DGE consumes ONE
                # index per partition ([128,1] offsets -> [128,C] rows)
                g_t = s4bp.tile([128, 4, cap, C], F16, tag="g")
                for a4 in range(4):
                    for j4 in range(cap):
                        nc.gpsimd.indirect_dma_start(
                            out=g_t[:, a4, j4, :], out_offset=None,
                            in_=table[:, :],
                            in_offset=bass.IndirectOffsetOnAxis(
                                ap=cols_t[:, a4, j4:j4 + 1], axis=0),
                        )
                xinT_t = s4p.tile([C, 512], F16, tag="xinT")
                nc.sync.dma_start(out=xinT_t[:],
                                  in_=x_inT[:, sp2 * 512:(sp2 + 1) * 512])
                oT_t = s4p.tile([C, 512], F32, tag="oT")

                if debug_taps and sp2 == 0:
                    nc.sync.dma_start(
                        out=dbg_g[:, :],
                        in_=g_t[:, 0:2, :, :].rearrange("p a j c -> p (a j c)"))
                for si in range(2):
                    s = 2 * sp2 + si
                    ssl = slice(s * 256, (s + 1) * 256)
                    r1 = s4p.tile([C, 512], F16, tag="r1")
                    r2 = s4p.tile([C, 512], F16, tag="r2")
                    for i in range(2):
                        blk = 2 * si + i
                        # S = [eq*vx | eq*vy] built per chunk in one fused
                        # op: (iota2 == rows[p]) * [vx[p] | vy[p]]
                        sp_t = s4bp.tile([128, cap, 256], F16, tag="sp")
                        mv = meta_t[:].rearrange(
                            "p a (three j) -> p a three j", three=3)
                        for j in range(cap):
                            nc.vector.scalar_tensor_tensor(
                                out=sp_t[:, j, :].rearrange(
                                    "p (two r) -> p two r", two=2),
                                in0=iota2[:],
                                scalar=meta_t[:, blk, j:j + 1],
                                in1=mv[:, blk, 1:3, j].to_broadcast(
                                    [128, 2, 128]),
                                op0=ALU.is_equal, op1=ALU.mult)
                        if debug_taps and s == 0 and i == 0:
                            nc.sync.dma_start(out=dbg_sp[:, :],
                                              in_=sp_t[:].rearrange("p j r -> p (j r)"))
                        pgxy = ps_gxy.tile([C, 256], F32, tag="gxy")
                        for j in range(cap):
                            nc.tensor.matmul(
                                pgxy[:], lhsT=g_t[:, blk, j, :],
                                rhs=sp_t[:, j, :],
                                start=(j == 0), stop=(j == cap - 1))
                        # r1 = [gxT | gyT], r2 = [gyT | -gxT] per block
                        nc.scalar.activation(out=r1[:, i * 256:(i + 1) * 256],
                                             in_=pgxy[:], func=AF.Copy)
                        nc.scalar.activation(out=r2[:, i * 256:i * 256 + 128],
                                             in_=pgxy[:, 128:256],
                                             func=AF.Copy)
                        nc.scalar.activation(
                            out=r2[:, i * 256 + 128:(i + 1) * 256],
                            in_=pgxy[:, 0:128], func=AF.Copy, scale=-1.0)

                    if debug_taps and s == 0:
                        nc.sync.dma_start(out=dbg_r1[:, :], in_=r1[:])
                    pvb = ps_vb.tile([C, 512], F32, tag="vb")
                    nc.tensor.matmul(pvb[:], lhsT=are_sb[:], rhs=r1[:],
                                     start=True, stop=False)
                    nc.tensor.matmul(pvb[:], lhsT=aimn_sb[:], rhs=r2[:],
                                     start=False, stop=True)

                    # x_grad^T = tanh(gx*vbre + gy*vbim)
                    pp = s4p.tile([C, 512], F16, tag="pp")
                    nc.vector.tensor_tensor(out=pp[:], in0=r1[:], in1=pvb[:],
                                            op=ALU.mult)
                    ppv = pp[:].rearrange("c (b two r) -> c b two r",
                                          two=2, r=128)
                    xg = s4p.tile([C, 2, 128], F16, tag="xg")
                    nc.gpsimd.tensor_tensor(out=xg[:], in0=ppv[:, :, 0, :],
                                            in1=ppv[:, :, 1, :], op=ALU.add)
                    xgt = s4p.tile([C, 256], F16, tag="xgt")
                    nc.scalar.activation(
                        out=xgt[:], in_=xg[:].rearrange("c b r -> c (b r)"),
                        func=AF.Tanh)
                    if debug_taps and s == 0:
                        nc.sync.dma_start(out=dbg_xgt[:, :], in_=xgt[:])

                    # MLP
                    xsl = slice(si * 256, (si + 1) * 256)
                    ph = ps_h.tile([H, 256], F32, tag="h")
                    nc.tensor.matmul(ph[:], lhsT=w0a_sb[:], rhs=xinT_t[:, xsl],
                                     start=True, stop=False)
                    nc.tensor.matmul(ph[:], lhsT=w0b_sb[:], rhs=xdT[:, ssl],
                                     start=False, stop=False)
                    nc.tensor.matmul(ph[:], lhsT=w0c_sb[:], rhs=xgt[:],
                                     start=False, stop=True)
                    hr = s4p.tile([H, 256], F16, tag="hr")
                    nc.scalar.activation(out=hr[:], in_=ph[:], func=AF.Relu,
                                         bias=b0_sb[:, :1])
                    if debug_taps and s == 0:
                        nc.sync.dma_start(out=dbg_hr[:, :], in_=hr[:])
                    po = ps_o.tile([C, 256], F32, tag="o")
                    nc.tensor.matmul(po[:], lhsT=w1_sb[:], rhs=hr[:],
                                     start=True, stop=True)
                    o1 = s4p.tile([C, 256], F32, tag="o1")
                    nc.scalar.activation(out=o1[:], in_=po[:],
                                         func=AF.Identity, bias=b1_sb[:, :1])
                    nc.vector.tensor_tensor(out=oT_t[:, xsl], in0=o1[:],
                                            in1=xinT_t[:, xsl], op=ALU.add)
                nc.sync.dma_start(out=outT[:, sp2 * 512:(sp2 + 1) * 512],
                                  in_=oT_t[:])

    _spill_waits(nc)
    return nc


# ---------------------------------------------------------------------------
# Host-side preprocessing
# ---------------------------------------------------------------------------
def host_prep(x_in, mass, L, evals, evecs, grad_rows, grad_cols,
              gradX_vals, gradY_vals, diffusion_time, A_re, A_im,
              W0, b0, W1, b1, cap=CAP_DEFAULT):
    """Build the 8 per-core input dicts. Returns (in_maps, cap_used)."""
    x_in = np.asarray(x_in, np.float32)
    mass = np.asarray(mass, np.float32)
    evals = np.asarray(evals, np.float32)
    evecs = np.asarray(evecs, np.float32)
    grad_rows = np.asarray(grad_rows)
    grad_cols = np.asarray(grad_cols)
    gradX_vals = np.asarray(gradX_vals, np.float32)
    gradY_vals = np.asarray(gradY_vals, np.float32)

    t = np.clip(np.asarray(diffusion_time, np.float32), 1e-8, None)
    W0 = np.asarray(W0, np.float32)
    W1 = np.asarray(W1, np.float32)
    b0 = np.asarray(b0, np.float32)
    b1 = np.asarray(b1, np.float32)
    A_re = np.asarray(A_re, np.float32)
    A_im = np.asarray(A_im, np.float32)

    need_cap = cap
    metas = []
    for b in range(B):
        r = grad_rows[b]
        for half in range(2):
            lo, hi = half * VH, (half + 1) * VH
            sel_e = (r >= lo) & (r < hi)
            cnt = np.bincount((r[sel_e] - lo) // 128, minlength=NBLK)
            need_cap = max(need_cap, int(np.ceil(cnt.max() / 128)))
            metas.append((b, half, sel_e))
    cap = int(need_cap)

    in_maps = []
    for b, half, sel_e in metas:
        lo = half * VH
        r = grad_rows[b][sel_e] - lo
        c = grad_cols[b][sel_e]
        vx = gradX_vals[b][sel_e]
        vy = gradY_vals[b][sel_e]
        order = np.argsort(r, kind="stable")
        r, c, vx, vy = r[order], c[order], vx[order], vy[order]

        # table row for a global col in this core's [own | other] table
        if half == 0:
            tc_ = np.where(c < VH, c, c + (VHP - VH))
        else:
            tc_ = np.where(c >= VH, c - VH, c + VHP)

        cols_a = np.zeros((NBLK, 128, cap), np.int32)
        rvxy_a = np.zeros((NBLK, 128, 3 * cap), np.float16)
        rvxy_a[:, :, 0:cap] = -1.0
        blk_of = r // 128
        starts = np.searchsorted(blk_of, np.arange(NBLK + 1))
        for blk in range(NBLK):
            s0, s1 = starts[blk], starts[blk + 1]
            n = s1 - s0
            assert n <= cap * 128
            j = np.arange(n) // 128
            p = np.arange(n) % 128
            cols_a[blk, p, j] = tc_[s0:s1]
            rvxy_a[blk, p, j] = (r[s0:s1] - blk * 128).astype(np.float16)
            rvxy_a[blk, p, cap + j] = vx[s0:s1].astype(np.float16)
            rvxy_a[blk, p, 2 * cap + j] = vy[s0:s1].astype(np.float16)

        xpad = np.zeros((VHP, C), np.float16)
        xpad[:VH] = x_in[b, lo:lo + VH]
        mpad = np.zeros((VHP, 1), np.float32)
        mpad[:VH, 0] = mass[b, lo:lo + VH]
        epad = np.zeros((VHP, K), np.float16)
        epad[:VH] = evecs[b, lo:lo + VH]
        oth = (1 - half) * VH
        evoT_a = np.zeros((K, VHP), np.float16)
        evoT_a[:, :VH] = evecs[b, oth:oth + VH].T
        evT_a = np.zeros((K, VHP), np.float16)
        evT_a[:, :VH] = evecs[b, lo:lo + VH].T
        x_inT_a = np.zeros((C, VHP), np.float16)
        x_inT_a[:, :VH] = x_in[b, lo:lo + VH].T

        decay = np.exp(-evals[b][:, None] * t[None, :]).astype(np.float32)

        in_maps.append({
            "x_in": xpad, "x_inT": x_inT_a, "mass": mpad,
            "ev_own": epad, "evT_own": evT_a, "evoT": evoT_a,
            "decay": decay,
            "a_re": A_re.astype(np.float16),
            "a_imn": (-A_im).astype(np.float16),
            "w0a": W0[0:C].astype(np.float16),
            "w0b": W0[C:2 * C].astype(np.float16),
            "w0c": W0[2 * C:3 * C].astype(np.float16),
            "w1": W1.astype(np.float16),
            "b0": b0.reshape(H, 1).astype(np.float32),
            "b1": b1.reshape(C, 1).astype(np.float32),
            "cols": cols_a, "rvxy": rvxy_a,
            "sel": np.repeat(np.eye(4, dtype=np.float32)[b][None, :], 128,
                             axis=0),
        })
    return in_maps, cap


_NC_CACHE = {}


def _get_nc(cap):
    if cap not in _NC_CACHE:
        _NC_CACHE[cap] = build_nc(cap=cap)
    return _NC_CACHE[cap]


def assemble(res) -> np.ndarray:
    out = np.empty((B, V, C), np.float32)
    for i in range(N_CORES):
        b, half = i // 2, i % 2
        out[b, half * VH:(half + 1) * VH] = res.results[i]["outT"].T[:VH]
    return out


def kernel(**inputs) -> np.ndarray:
    in_maps, cap = host_prep(**inputs)
    nc = _get_nc(cap)
    res = run_bass_kernel_spmd(nc, in_maps, list(range(N_CORES)))
    return assemble(res)



# revision 9
# speedup vs baseline: 2.6516x; 2.6516x over previous
"""DiffusionNetBlock Bass/Tile kernel for 8 trn2 NeuronCores.

Sharding: 2 cores per batch sample (B=4); each core owns half the
vertices (rows). The spectral projection is computed over the own half
and summed across the pair with one 8-core AllReduce (4 sample slots).
Each core builds the FULL fp16 x_diffuse gather table locally
(replicated stage-2 work beats a 2-rank collective). Sparse gradient
SpMM: edges are bucketed by destination 128-row block (sorted on host),
padded to CAP chunks of 128 edges; cols are gathered from the local
table via one indirect DMA PER 128-EDGE CHUNK ([128,1] offsets -> one
row per partition; TRN2 HW consumes exactly one index per partition,
unlike CoreSim); the segment sum is a one-hot
matmul per chunk (lhsT = gathered g, rhs = [eq*vx | eq*vy]) accumulating
gx^T|gy^T in PSUM. Rotation, tanh, MLP and residual are fused per
256-row superblock in [feature, vertex] layout; the host transposes the
output back. evecs^T / x_in^T come pre-transposed (fp16) from the host.
"""
import sys

for _p in ("/opt/trn_rl_repo", "/opt/pypackages"):
    if _p not in sys.path:
        sys.path.append(_p)

from contextlib import ExitStack

import numpy as np

import bass_rust
import concourse.bass as bass
import concourse.mybir as mybir
import concourse.tile as tile
from concourse.bass_utils import run_bass_kernel_spmd
from concourse.masks import make_identity
from concourse.vector_clock import ScopedClock

F32 = mybir.dt.float32
F16 = mybir.dt.float16
I32 = mybir.dt.int32
I16 = mybir.dt.int16
AF = mybir.ActivationFunctionType
ALU = mybir.AluOpType

# Problem dims (hardcoded per spec)
B, V, C, K, E, H = 4, 50000, 128, 128, 400000, 128
N_CORES = 8
VH = V // 2            # 25000 vertices per core
NBLK = (VH + 127) // 128   # 196 blocks of 128 rows
VHP = NBLK * 128       # 25088 padded
NSUP = NBLK // 2       # 98 superblocks of 256 rows
CAP_DEFAULT = 10       # 128-edge chunks per block (mean 8, 8-sigma margin)


# ---------------------------------------------------------------------------
# Tile drain patch: walrus in this container rejects CTRL (Drain)
# instructions carrying sem waits. Move the kernel-tail drain's waits onto
# single-wait NOPs (sequential issue on SP makes the chain equivalent).
# ---------------------------------------------------------------------------
_MAX_WAIT_NOPS = 48


def _drain_and_barrier_split(self, tick_clock, wait_clock):
    nc = self.nc
    nops = [
        nc.sync.nop(nofuse=True, hint=f"drain_wait_split_{i}")
        for i in range(_MAX_WAIT_NOPS)
    ]
    drain_inst = nc.sync.drain().ins
    wait_clock.add_sem_waits(drain_inst, ScopedClock({None: tick_clock.global_clock}))
    si = drain_inst.sync_info
    waits = list(si.on_wait) if si is not None else []
    assert len(waits) <= _MAX_WAIT_NOPS, f"too many drain waits: {len(waits)}"
    if waits:
        drain_inst.sync_info = bass_rust.SyncInfo(
            on_wait=[], on_update=list(si.on_update)
        )
        for w, nop in zip(waits, nops):
            nop.ins.sync_info = bass_rust.SyncInfo(on_wait=[w], on_update=[])

    nc.all_engine_barrier()
    assert self.sems is not None
    popped = nc._tile_sem_poison_stack.pop()
    assert popped is self._sem_poison
    nc.clear_and_free_semaphores(list(self.sems.allocated().values()))
    nc.all_engine_barrier()


tile.TileContext._drain_and_barrier = _drain_and_barrier_split

_spill_id = [0]


def _spill_waits(nc, cap=1, drain_cap=1):
    """Walrus in this container bounds sem-waits per instruction (CTRL ~1,
    compute/DMA ~2). Move excess waits onto same-engine NOPs inserted right
    before the instruction (engines issue in order, so a chain of
    single-wait NOPs is equivalent to one multi-wait instruction)."""
    for f in nc.m.functions:
        for bb in f.blocks:
            out = []
            changed = False
            for inst in bb.instructions:
                si = inst.sync_info
                waits = list(si.on_wait) if si is not None else []
                c = drain_cap if isinstance(inst, mybir.InstDrain) else cap
                if len(waits) > c:
                    keep = waits[len(waits) - c:] if c else []
                    for w in waits[:len(waits) - c]:
                        nop = mybir.InstNoOp(
                            name=f"wspill-{_spill_id[0]}", ins=[], outs=[])
                        _spill_id[0] += 1
                        nop.engine = inst.engine
                        nop.sync_info = bass_rust.SyncInfo(
                            on_wait=[w], on_update=[])
                        nc.register_instruction(nop, overwrite=True)
                        out.append(nop)
                    inst.sync_info = bass_rust.SyncInfo(
                        on_wait=keep, on_update=list(si.on_update))
                    changed = True
                out.append(inst)
            if changed:
                bb.instructions = out


# ---------------------------------------------------------------------------
# Program builder
# ---------------------------------------------------------------------------
def build_nc(nblk=NBLK, cap=CAP_DEFAULT, n_cores=N_CORES, debug_taps=False):
    """Build the SPMD Bass program. All per-core variation is in the data."""
    vhp = nblk * 128
    nsup = nblk // 2
    assert nblk % 4 == 0
    tvp = 2 * vhp  # gather table rows
    nq = nblk // 4  # quad count for batched streaming

    nc = bass.Bass("TRN2", target_bir_lowering=False, debug=False,
                   num_devices=n_cores)

    def din(name, shape, dt):
        return nc.dram_tensor(name, shape, dt, kind="ExternalInput").ap()

    x_in = din("x_in", [vhp, C], F16)        # stage-1 stream
    x_inT = din("x_inT", [C, vhp], F16)      # MLP rhs + residual
    mass = din("mass", [vhp, 1], F32)
    ev_own = din("ev_own", [vhp, K], F16)    # stage-1 lhsT stream
    evT_own = din("evT_own", [K, vhp], F16)  # resident, stage-2 own
    evoT = din("evoT", [K, vhp], F16)        # stage-2 other-half stream
    decay = din("decay", [K, C], F32)
    a_re = din("a_re", [C, C], F16)
    a_imn = din("a_imn", [C, C], F16)   # -A_im
    w0a = din("w0a", [C, H], F16)
    w0b = din("w0b", [C, H], F16)
    w0c = din("w0c", [C, H], F16)
    w1 = din("w1", [H, C], F16)
    b0 = din("b0", [H, 1], F32)
    b1 = din("b1", [C, 1], F32)
    cols = din("cols", [nblk, 128, cap], I32)
    # rvxy packs rows | vx | vy along the last axis
    rvxy = din("rvxy", [nblk, 128, 3 * cap], F16)
    # sel[s] = 1.0 iff this core's sample is slot s (same program, data-driven)
    sel = din("sel", [128, 4], F32)

    outT = nc.dram_tensor("outT", [C, vhp], F32, kind="ExternalOutput").ap()
    if debug_taps:
        dbg_xs = nc.dram_tensor("dbg_xs", [K, C], F32, kind="ExternalOutput").ap()
        dbg_tab = nc.dram_tensor("dbg_tab", [1024, C], F16, kind="ExternalOutput").ap()
        dbg_r1 = nc.dram_tensor("dbg_r1", [C, 512], F16, kind="ExternalOutput").ap()
        dbg_xgt = nc.dram_tensor("dbg_xgt", [C, 256], F16, kind="ExternalOutput").ap()
        dbg_hr = nc.dram_tensor("dbg_hr", [H, 256], F16, kind="ExternalOutput").ap()
        dbg_g = nc.dram_tensor("dbg_g", [128, 2 * cap * C], F16, kind="ExternalOutput").ap()
        dbg_sp = nc.dram_tensor("dbg_sp", [128, cap * 256], F16, kind="ExternalOutput").ap()

    nslot = 4
    xs_loc = nc.dram_tensor("xs_loc", [nslot * K, C], F32).ap()
    xs_sh = nc.dram_tensor("xs_sh", [nslot * K, C], F32,
                           addr_space="Shared").ap()
    table = nc.dram_tensor("table", [tvp, C], F16).ap()

    groups = [list(range(n_cores))]

    with tile.TileContext(nc) as tc, ExitStack() as ctx:
        cpool = ctx.enter_context(tc.tile_pool(name="const", bufs=1))
        rpool = ctx.enter_context(tc.tile_pool(name="resid", bufs=1))

        # iota over r, twice side by side: [128, 2, 128] (for [Sx | Sy])
        iota_i = cpool.tile([128, 2, 128], I16, tag="iota_i")
        nc.gpsimd.iota(iota_i[:], pattern=[[0, 2], [1, 128]], base=0,
                       channel_multiplier=0)
        iota2 = cpool.tile([128, 2, 128], F16, tag="iota2")
        nc.vector.tensor_copy(out=iota2[:], in_=iota_i[:])

        # constants
        decay_sb = cpool.tile([K, C], F32, tag="decay")
        nc.sync.dma_start(out=decay_sb[:], in_=decay[:, :])
        are_sb = cpool.tile([C, C], F16, tag="are")
        nc.sync.dma_start(out=are_sb[:], in_=a_re[:, :])
        aimn_sb = cpool.tile([C, C], F16, tag="aimn")
        nc.sync.dma_start(out=aimn_sb[:], in_=a_imn[:, :])
        w0a_sb = cpool.tile([C, H], F16, tag="w0a")
        nc.sync.dma_start(out=w0a_sb[:], in_=w0a[:, :])
        w0b_sb = cpool.tile([C, H], F16, tag="w0b")
        nc.sync.dma_start(out=w0b_sb[:], in_=w0b[:, :])
        w0c_sb = cpool.tile([C, H], F16, tag="w0c")
        nc.sync.dma_start(out=w0c_sb[:], in_=w0c[:, :])
        w1_sb = cpool.tile([H, C], F16, tag="w1")
        nc.sync.dma_start(out=w1_sb[:], in_=w1[:, :])
        b0_sb = cpool.tile([H, 1], F32, tag="b0")
        nc.sync.dma_start(out=b0_sb[:], in_=b0[:, :])
        b1_sb = cpool.tile([C, 1], F32, tag="b1")
        nc.sync.dma_start(out=b1_sb[:], in_=b1[:, :])
        sel_sb = cpool.tile([128, 4], F32, tag="sel")
        nc.sync.dma_start(out=sel_sb[:], in_=sel[:, :])
        mass_sb = cpool.tile([128, nblk], F32, tag="mass")
        nc.sync.dma_start(out=mass_sb[:],
                          in_=mass[:, :].rearrange("(t p) one -> p (t one)",
                                                   p=128))

        # residents
        evT = rpool.tile([K, vhp], F16, tag="evT")
        nc.sync.dma_start(out=evT[:], in_=evT_own[:, :])
        xdT = rpool.tile([C, vhp], F16, tag="xdT")

        # ---------------- stage 1: spectral projection ----------------
        with tc.tile_pool(name="s1", bufs=3) as s1p, \
             tc.tile_pool(name="ps_xs", bufs=1, space="PSUM") as ps_xs:
            psum_xs = ps_xs.tile([K, C], F32, tag="xs")
            for q in range(nq):
                qsl = slice(q * 512, (q + 1) * 512)
                xin_q = s1p.tile([128, 4, C], F16, tag="xin")
                ev_q = s1p.tile([128, 4, K], F16, tag="ev")
                nc.sync.dma_start(
                    out=xin_q[:],
                    in_=x_in[qsl, :].rearrange("(a p) c -> p a c", p=128))
                nc.sync.dma_start(
                    out=ev_q[:],
                    in_=ev_own[qsl, :].rearrange("(a p) c -> p a c", p=128))
                mx_q = s1p.tile([128, 4, C], F16, tag="mx")
                for a in range(4):
                    t = 4 * q + a
                    nc.vector.tensor_scalar_mul(mx_q[:, a, :], xin_q[:, a, :],
                                                mass_sb[:, t:t + 1])
                    nc.tensor.matmul(psum_xs[:], lhsT=ev_q[:, a, :],
                                     rhs=mx_q[:, a, :],
                                     start=(t == 0), stop=(t == nblk - 1))

            xs_sb = cpool.tile([K, C], F32, tag="xs_sb")
            nc.scalar.activation(out=xs_sb[:], in_=psum_xs[:], func=AF.Copy)
            for s in range(nslot):
                slot_sb = s1p.tile([K, C], F32, tag="slot")
                nc.vector.tensor_scalar_mul(slot_sb[:], xs_sb[:],
                                            sel_sb[:, s:s + 1])
                nc.sync.dma_start(out=xs_loc[s * K:(s + 1) * K, :],
                                  in_=slot_sb[:])

        nc.gpsimd.collective_compute(
            "AllReduce", ALU.add,
            ins=[xs_loc[:, :]], outs=[xs_sh[:, :]],
            replica_groups=groups,
        )

        # xs = decay * (own slot of the AllReduce result)
        slots_sb = cpool.tile([K, nslot * C], F32, tag="slots")
        for s in range(nslot):
            nc.sync.dma_start(out=slots_sb[:, s * C:(s + 1) * C],
                              in_=xs_sh[s * K:(s + 1) * K, :])
        xs_ar = cpool.tile([K, C], F32, tag="xs_ar")
        nc.vector.tensor_scalar_mul(xs_ar[:], slots_sb[:, 0:C],
                                    sel_sb[:, 0:1])
        for s in range(1, nslot):
            nc.vector.scalar_tensor_tensor(
                out=xs_ar[:], in0=slots_sb[:, s * C:(s + 1) * C],
                scalar=sel_sb[:, s:s + 1], in1=xs_ar[:],
                op0=ALU.mult, op1=ALU.add)
        xs32 = cpool.tile([K, C], F32, tag="xs32")
        nc.vector.tensor_tensor(out=xs32[:], in0=xs_ar[:], in1=decay_sb[:],
                                op=ALU.mult)
        xs16 = cpool.tile([K, C], F16, tag="xs16")
        nc.vector.tensor_copy(out=xs16[:], in_=xs32[:])
        if debug_taps:
            nc.sync.dma_start(out=dbg_xs[:, :], in_=xs32[:])

        # ---------------- stage 2: x_diffuse table + xdT ----------------
        with tc.tile_pool(name="s2", bufs=3) as s2p, \
             tc.tile_pool(name="ps_xd", bufs=3, space="PSUM") as ps_xd, \
             tc.tile_pool(name="ps_xt", bufs=3, space="PSUM") as ps_xt:
            for q in range(nq):
                td_q = s2p.tile([128, 4, C], F16, tag="td")
                for a in range(4):
                    t = 4 * q + a
                    vsl = slice(t * 128, (t + 1) * 128)
                    pxd = ps_xd.tile([128, C], F32, tag="xd")
                    nc.tensor.matmul(pxd[:], lhsT=evT[:, vsl], rhs=xs16[:],
                                     start=True, stop=True)
                    nc.scalar.activation(out=td_q[:, a, :], in_=pxd[:],
                                         func=AF.Copy)
                    pxt = ps_xt.tile([C, 128], F32, tag="xdt")
                    nc.tensor.matmul(pxt[:], lhsT=xs16[:], rhs=evT[:, vsl],
                                     start=True, stop=True)
                    nc.vector.tensor_copy(out=xdT[:, vsl], in_=pxt[:])
                nc.sync.dma_start(
                    out=table[q * 512:(q + 1) * 512, :].rearrange(
                        "(a p) c -> p a c", p=128),
                    in_=td_q[:])
                if debug_taps and q == 0:
                    nc.sync.dma_start(
                        out=dbg_tab[0:512, :].rearrange("(a p) c -> p a c", p=128),
                        in_=td_q[:])

            for q in range(nq):
                evo_q = s2p.tile([128, 4, 128], F16, tag="evo")
                nc.sync.dma_start(
                    out=evo_q[:],
                    in_=evoT[:, q * 512:(q + 1) * 512].rearrange(
                        "p (a c) -> p a c", a=4))
                td_q = s2p.tile([128, 4, C], F16, tag="td")
                for a in range(4):
                    pxd = ps_xd.tile([128, C], F32, tag="xd")
                    nc.tensor.matmul(pxd[:], lhsT=evo_q[:, a, :], rhs=xs16[:],
                                     start=True, stop=True)
                    nc.scalar.activation(out=td_q[:, a, :], in_=pxd[:],
                                         func=AF.Copy)
                nc.sync.dma_start(
                    out=table[vhp + q * 512:vhp + (q + 1) * 512, :].rearrange(
                        "(a p) c -> p a c", p=128),
                    in_=td_q[:])

        # ---------------- stages 3-5: SpMM + rotation + MLP ----------------
        with tc.tile_pool(name="s4", bufs=3) as s4p, \
             tc.tile_pool(name="s4b", bufs=3) as s4bp, \
             tc.tile_pool(name="gch", bufs=3 * cap) as gchp, \
             tc.tile_pool(name="ps_gxy", bufs=2, space="PSUM") as ps_gxy, \
             tc.tile_pool(name="ps_vb", bufs=2, space="PSUM") as ps_vb, \
             tc.tile_pool(name="ps_h", bufs=2, space="PSUM") as ps_h, \
             tc.tile_pool(name="ps_o", bufs=2, space="PSUM") as ps_o:
            for sp2 in range(nsup // 2):
                # batched loads + per-chunk gathers for 2 superblocks (4 blks)
                cols_t = s4p.tile([128, 4, cap], I32, tag="cols")
                nc.sync.dma_start(
                    out=cols_t[:],
                    in_=cols[4 * sp2:4 * sp2 + 4].rearrange("a p j -> p a j"))
                meta_t = s4p.tile([128, 4, 3 * cap], F16, tag="meta")
                nc.sync.dma_start(
                    out=meta_t[:],
                    in_=rvxy[4 * sp2:4 * sp2 + 4].rearrange("a p j -> p a j"))
                # one indirect DMA per 128-edge chunk: the DGE consumes ONE
                # index per partition ([128,1] offsets -> [128,C] rows).
                # Separate per-chunk tiles (deep pool) so successive gathers
                # pipeline instead of WAW-serializing on one big tile, and so
                # each consuming matmul waits only for its own chunk.
                g_ch = [[None] * cap for _ in range(4)]
                for a4 in range(4):
                    for j4 in range(cap):
                        gt = gchp.tile([128, C], F16, tag="gch")
                        nc.gpsimd.indirect_dma_start(
                            out=gt[:], out_offset=None,
                            in_=table[:, :],
                            in_offset=bass.IndirectOffsetOnAxis(
                                ap=cols_t[:, a4, j4:j4 + 1], axis=0),
                        )
                        g_ch[a4][j4] = gt
                xinT_t = s4p.tile([C, 512], F16, tag="xinT")
                nc.sync.dma_start(out=xinT_t[:],
                                  in_=x_inT[:, sp2 * 512:(sp2 + 1) * 512])
                oT_t = s4p.tile([C, 512], F32, tag="oT")

                if debug_taps and sp2 == 0:
                    for a4 in range(2):
                        for j4 in range(cap):
                            nc.sync.dma_start(
                                out=dbg_g[:, (a4 * cap + j4) * C:
                                          (a4 * cap + j4 + 1) * C],
                                in_=g_ch[a4][j4][:])
                for si in range(2):
                    s = 2 * sp2 + si
                    ssl = slice(s * 256, (s + 1) * 256)
                    r1 = s4p.tile([C, 512], F16, tag="r1")
                    r2 = s4p.tile([C, 512], F16, tag="r2")
                    for i in range(2):
                        blk = 2 * si + i
                        # S = [eq*vx | eq*vy] built per chunk in one fused
                        # op: (iota2 == rows[p]) * [vx[p] | vy[p]]
                        sp_t = s4bp.tile([128, cap, 256], F16, tag="sp")
                        mv = meta_t[:].rearrange(
                            "p a (three j) -> p a three j", three=3)
                        for j in range(cap):
                            nc.vector.scalar_tensor_tensor(
                                out=sp_t[:, j, :].rearrange(
                                    "p (two r) -> p two r", two=2),
                                in0=iota2[:],
                                scalar=meta_t[:, blk, j:j + 1],
                                in1=mv[:, blk, 1:3, j].to_broadcast(
                                    [128, 2, 128]),
                                op0=ALU.is_equal, op1=ALU.mult)
                        if debug_taps and s == 0 and i == 0:
                            nc.sync.dma_start(out=dbg_sp[:, :],
                                              in_=sp_t[:].rearrange("p j r -> p (j r)"))
                        pgxy = ps_gxy.tile([C, 256], F32, tag="gxy")
                        for j in range(cap):
                            nc.tensor.matmul(
                                pgxy[:], lhsT=g_ch[blk][j][:],
                                rhs=sp_t[:, j, :],
                                start=(j == 0), stop=(j == cap - 1))
                        # r1 = [gxT | gyT], r2 = [gyT | -gxT] per block
                        nc.scalar.activation(out=r1[:, i * 256:(i + 1) * 256],
                                             in_=pgxy[:], func=AF.Copy)
                        nc.scalar.activation(out=r2[:, i * 256:i * 256 + 128],
                                             in_=pgxy[:, 128:256],
                                             func=AF.Copy)
                        nc.scalar.activation(
                            out=r2[:, i * 256 + 128:(i + 1) * 256],
                            in_=pgxy[:, 0:128], func=AF.Copy, scale=-1.0)

                    if debug_taps and s == 0:
                        nc.sync.dma_start(out=dbg_r1[:, :], in_=r1[:])
                    pvb = ps_vb.tile([C, 512], F32, tag="vb")
                    nc.tensor.matmul(pvb[:], lhsT=are_sb[:], rhs=r1[:],
                                     start=True, stop=False)
                    nc.tensor.matmul(pvb[:], lhsT=aimn_sb[:], rhs=r2[:],
                                     start=False, stop=True)

                    # x_grad^T = tanh(gx*vbre + gy*vbim)
                    pp = s4p.tile([C, 512], F16, tag="pp")
                    nc.vector.tensor_tensor(out=pp[:], in0=r1[:], in1=pvb[:],
                                            op=ALU.mult)
                    ppv = pp[:].rearrange("c (b two r) -> c b two r",
                                          two=2, r=128)
                    # keep this off gpsimd: the Pool queue also issues every
                    # indirect gather, so compute there stalls the gathers
                    xg = s4p.tile([C, 2, 128], F16, tag="xg")
                    nc.vector.tensor_tensor(out=xg[:], in0=ppv[:, :, 0, :],
                                            in1=ppv[:, :, 1, :], op=ALU.add)
                    xgt = s4p.tile([C, 256], F16, tag="xgt")
                    nc.scalar.activation(
                        out=xgt[:], in_=xg[:].rearrange("c b r -> c (b r)"),
                        func=AF.Tanh)
                    if debug_taps and s == 0:
                        nc.sync.dma_start(out=dbg_xgt[:, :], in_=xgt[:])

                    # MLP
                    xsl = slice(si * 256, (si + 1) * 256)
                    ph = ps_h.tile([H, 256], F32, tag="h")
                    nc.tensor.matmul(ph[:], lhsT=w0a_sb[:], rhs=xinT_t[:, xsl],
                                     start=True, stop=False)
                    nc.tensor.matmul(ph[:], lhsT=w0b_sb[:], rhs=xdT[:, ssl],
                                     start=False, stop=False)
                    nc.tensor.matmul(ph[:], lhsT=w0c_sb[:], rhs=xgt[:],
                                     start=False, stop=True)
                    hr = s4p.tile([H, 256], F16, tag="hr")
                    nc.scalar.activation(out=hr[:], in_=ph[:], func=AF.Relu,
                                         bias=b0_sb[:, :1])
                    if debug_taps and s == 0:
                        nc.sync.dma_start(out=dbg_hr[:, :], in_=hr[:])
                    po = ps_o.tile([C, 256], F32, tag="o")
                    nc.tensor.matmul(po[:], lhsT=w1_sb[:], rhs=hr[:],
                                     start=True, stop=True)
                    o1 = s4p.tile([C, 256], F32, tag="o1")
                    nc.scalar.activation(out=o1[:], in_=po[:],
                                         func=AF.Identity, bias=b1_sb[:, :1])
                    nc.vector.tensor_tensor(out=oT_t[:, xsl], in0=o1[:],
                                            in1=xinT_t[:, xsl], op=ALU.add)
                nc.sync.dma_start(out=outT[:, sp2 * 512:(sp2 + 1) * 512],
                                  in_=oT_t[:])

    _spill_waits(nc)
    return nc


# ---------------------------------------------------------------------------
# Host-side preprocessing
# ---------------------------------------------------------------------------
def host_prep(x_in, mass, L, evals, evecs, grad_rows, grad_cols,
              gradX_vals, gradY_vals, diffusion_time, A_re, A_im,
              W0, b0, W1, b1, cap=CAP_DEFAULT):
    """Build the 8 per-core input dicts. Returns (in_maps, cap_used)."""
    x_in = np.asarray(x_in, np.float32)
    mass = np.asarray(mass, np.float32)
    evals = np.asarray(evals, np.float32)
    evecs = np.asarray(evecs, np.float32)
    grad_rows = np.asarray(grad_rows)
    grad_cols = np.asarray(grad_cols)
    gradX_vals = np.asarray(gradX_vals, np.float32)
    gradY_vals = np.asarray(gradY_vals, np.float32)

    t = np.clip(np.asarray(diffusion_time, np.float32), 1e-8, None)
    W0 = np.asarray(W0, np.float32)
    W1 = np.asarray(W1, np.float32)
    b0 = np.asarray(b0, np.float32)
    b1 = np.asarray(b1, np.float32)
    A_re = np.asarray(A_re, np.float32)
    A_im = np.asarray(A_im, np.float32)

    need_cap = cap
    metas = []
    for b in range(B):
        r = grad_rows[b]
        for half in range(2):
            lo, hi = half * VH, (half + 1) * VH
            sel_e = (r >= lo) & (r < hi)
            cnt = np.bincount((r[sel_e] - lo) // 128, minlength=NBLK)
            need_cap = max(need_cap, int(np.ceil(cnt.max() / 128)))
            metas.append((b, half, sel_e))
    cap = int(need_cap)

    in_maps = []
    for b, half, sel_e in metas:
        lo = half * VH
        r = grad_rows[b][sel_e] - lo
        c = grad_cols[b][sel_e]
        vx = gradX_vals[b][sel_e]
        vy = gradY_vals[b][sel_e]
        order = np.argsort(r, kind="stable")
        r, c, vx, vy = r[order], c[order], vx[order], vy[order]

        # table row for a global col in this core's [own | other] table
        if half == 0:
            tc_ = np.where(c < VH, c, c + (VHP - VH))
        else:
            tc_ = np.where(c >= VH, c - VH, c + VHP)

        cols_a = np.zeros((NBLK, 128, cap), np.int32)
        rvxy_a = np.zeros((NBLK, 128, 3 * cap), np.float16)
        rvxy_a[:, :, 0:cap] = -1.0
        blk_of = r // 128
        starts = np.searchsorted(blk_of, np.arange(NBLK + 1))
        for blk in range(NBLK):
            s0, s1 = starts[blk], starts[blk + 1]
            n = s1 - s0
            assert n <= cap * 128
            j = np.arange(n) // 128
            p = np.arange(n) % 128
            cols_a[blk, p, j] = tc_[s0:s1]
            rvxy_a[blk, p, j] = (r[s0:s1] - blk * 128).astype(np.float16)
            rvxy_a[blk, p, cap + j] = vx[s0:s1].astype(np.float16)
            rvxy_a[blk, p, 2 * cap + j] = vy[s0:s1].astype(np.float16)

        xpad = np.zeros((VHP, C), np.float16)
        xpad[:VH] = x_in[b, lo:lo + VH]
        mpad = np.zeros((VHP, 1), np.float32)
        mpad[:VH, 0] = mass[b, lo:lo + VH]
        epad = np.zeros((VHP, K), np.float16)
        epad[:VH] = evecs[b, lo:lo + VH]
        oth = (1 - half) * VH
        evoT_a = np.zeros((K, VHP), np.float16)
        evoT_a[:, :VH] = evecs[b, oth:oth + VH].T
        evT_a = np.zeros((K, VHP), np.float16)
        evT_a[:, :VH] = evecs[b, lo:lo + VH].T
        x_inT_a = np.zeros((C, VHP), np.float16)
        x_inT_a[:, :VH] = x_in[b, lo:lo + VH].T

        decay = np.exp(-evals[b][:, None] * t[None, :]).astype(np.float32)

        in_maps.append({
            "x_in": xpad, "x_inT": x_inT_a, "mass": mpad,
            "ev_own": epad, "evT_own": evT_a, "evoT": evoT_a,
            "decay": decay,
            "a_re": A_re.astype(np.float16),
            "a_imn": (-A_im).astype(np.float16),
            "w0a": W0[0:C].astype(np.float16),
            "w0b": W0[C:2 * C].astype(np.float16),
            "w0c": W0[2 * C:3 * C].astype(np.float16),
            "w1": W1.astype(np.float16),
            "b0": b0.reshape(H, 1).astype(np.float32),
            "b1": b1.reshape(C, 1).astype(np.float32),
            "cols": cols_a, "rvxy": rvxy_a,
            "sel": np.repeat(np.eye(4, dtype=np.float32)[b][None, :], 128,
                             axis=0),
        })
    return in_maps, cap


_NC_CACHE = {}


def _get_nc(cap):
    if cap not in _NC_CACHE:
        _NC_CACHE[cap] = build_nc(cap=cap)
    return _NC_CACHE[cap]


def assemble(res) -> np.ndarray:
    out = np.empty((B, V, C), np.float32)
    for i in range(N_CORES):
        b, half = i // 2, i % 2
        out[b, half * VH:(half + 1) * VH] = res.results[i]["outT"].T[:VH]
    return out


def kernel(**inputs) -> np.ndarray:
    in_maps, cap = host_prep(**inputs)
    nc = _get_nc(cap)
    res = run_bass_kernel_spmd(nc, in_maps, list(range(N_CORES)))
    return assemble(res)



# revision 10
# speedup vs baseline: 29.3140x; 11.0553x over previous
"""DiffusionNetBlock Bass/Tile kernel for 8 trn2 NeuronCores.

Sharding: 2 cores per batch sample (B=4); each core owns half the
vertices (rows). The spectral projection is computed over the own half
and summed across the pair with one 8-core AllReduce (4 sample slots).
Each core builds the FULL fp16 x_diffuse gather table locally
(replicated stage-2 work beats a 2-rank collective). Sparse gradient
SpMM: edges are bucketed by destination 128-row block (sorted on host),
padded to CAP chunks of 128 edges; cols are gathered from the local
table via one indirect DMA PER 128-EDGE CHUNK ([128,1] offsets -> one
row per partition; TRN2 HW consumes exactly one index per partition,
unlike CoreSim). Each chunk gathers into its OWN tile from a deep pool
(bufs=3*cap) so the ~2k SWDGE ops pipeline on qPoolDynamic instead of
WAW-serializing on one big tile (9.4ms -> 3.6ms), and no compute is
placed on gpsimd (it would stall the gather queue). The segment sum is
a one-hot matmul per chunk (lhsT = gathered g, rhs = [eq*vx | eq*vy])
accumulating gx^T|gy^T in PSUM. Rotation, tanh, MLP and residual are fused per
256-row superblock in [feature, vertex] layout; the host transposes the
output back. evecs^T / x_in^T come pre-transposed (fp16) from the host.
"""
import sys

for _p in ("/opt/trn_rl_repo", "/opt/pypackages"):
    if _p not in sys.path:
        sys.path.append(_p)

from contextlib import ExitStack

import numpy as np

import bass_rust
import concourse.bass as bass
import concourse.mybir as mybir
import concourse.tile as tile
from concourse.bass_utils import run_bass_kernel_spmd
from concourse.masks import make_identity
from concourse.vector_clock import ScopedClock

F32 = mybir.dt.float32
F16 = mybir.dt.float16
I32 = mybir.dt.int32
I16 = mybir.dt.int16
AF = mybir.ActivationFunctionType
ALU = mybir.AluOpType

# Problem dims (hardcoded per spec)
B, V, C, K, E, H = 4, 50000, 128, 128, 400000, 128
N_CORES = 8
VH = V // 2            # 25000 vertices per core
NBLK = (VH + 127) // 128   # 196 blocks of 128 rows
VHP = NBLK * 128       # 25088 padded
NSUP = NBLK // 2       # 98 superblocks of 256 rows
CAP_DEFAULT = 10       # 128-edge chunks per block (mean 8, 8-sigma margin)


# ---------------------------------------------------------------------------
# Tile drain patch: walrus in this container rejects CTRL (Drain)
# instructions carrying sem waits. Move the kernel-tail drain's waits onto
# single-wait NOPs (sequential issue on SP makes the chain equivalent).
# ---------------------------------------------------------------------------
_MAX_WAIT_NOPS = 48


def _drain_and_barrier_split(self, tick_clock, wait_clock):
    nc = self.nc
    nops = [
        nc.sync.nop(nofuse=True, hint=f"drain_wait_split_{i}")
        for i in range(_MAX_WAIT_NOPS)
    ]
    drain_inst = nc.sync.drain().ins
    wait_clock.add_sem_waits(drain_inst, ScopedClock({None: tick_clock.global_clock}))
    si = drain_inst.sync_info
    waits = list(si.on_wait) if si is not None else []
    assert len(waits) <= _MAX_WAIT_NOPS, f"too many drain waits: {len(waits)}"
    if waits:
        drain_inst.sync_info = bass_rust.SyncInfo(
            on_wait=[], on_update=list(si.on_update)
        )
        for w, nop in zip(waits, nops):
            nop.ins.sync_info = bass_rust.SyncInfo(on_wait=[w], on_update=[])

    nc.all_engine_barrier()
    assert self.sems is not None
    popped = nc._tile_sem_poison_stack.pop()
    assert popped is self._sem_poison
    nc.clear_and_free_semaphores(list(self.sems.allocated().values()))
    nc.all_engine_barrier()


tile.TileContext._drain_and_barrier = _drain_and_barrier_split

_spill_id = [0]


def _spill_waits(nc, cap=1, drain_cap=1):
    """Walrus in this container bounds sem-waits per instruction (CTRL ~1,
    compute/DMA ~2). Move excess waits onto same-engine NOPs inserted right
    before the instruction (engines issue in order, so a chain of
    single-wait NOPs is equivalent to one multi-wait instruction)."""
    for f in nc.m.functions:
        for bb in f.blocks:
            out = []
            changed = False
            for inst in bb.instructions:
                si = inst.sync_info
                waits = list(si.on_wait) if si is not None else []
                c = drain_cap if isinstance(inst, mybir.InstDrain) else cap
                if len(waits) > c:
                    keep = waits[len(waits) - c:] if c else []
                    for w in waits[:len(waits) - c]:
                        nop = mybir.InstNoOp(
                            name=f"wspill-{_spill_id[0]}", ins=[], outs=[])
                        _spill_id[0] += 1
                        nop.engine = inst.engine
                        nop.sync_info = bass_rust.SyncInfo(
                            on_wait=[w], on_update=[])
                        nc.register_instruction(nop, overwrite=True)
                        out.append(nop)
                    inst.sync_info = bass_rust.SyncInfo(
                        on_wait=keep, on_update=list(si.on_update))
                    changed = True
                out.append(inst)
            if changed:
                bb.instructions = out


# ---------------------------------------------------------------------------
# Program builder
# ---------------------------------------------------------------------------
def build_nc(nblk=NBLK, cap=CAP_DEFAULT, n_cores=N_CORES, debug_taps=False):
    """Build the SPMD Bass program. All per-core variation is in the data."""
    vhp = nblk * 128
    nsup = nblk // 2
    assert nblk % 4 == 0
    tvp = 2 * vhp  # gather table rows
    nq = nblk // 4  # quad count for batched streaming

    nc = bass.Bass("TRN2", target_bir_lowering=False, debug=False,
                   num_devices=n_cores)

    def din(name, shape, dt):
        return nc.dram_tensor(name, shape, dt, kind="ExternalInput").ap()

    x_in = din("x_in", [vhp, C], F16)        # stage-1 stream
    x_inT = din("x_inT", [C, vhp], F16)      # MLP rhs + residual
    mass = din("mass", [vhp, 1], F32)
    ev_own = din("ev_own", [vhp, K], F16)    # stage-1 lhsT stream
    evT_own = din("evT_own", [K, vhp], F16)  # resident, stage-2 own
    evoT = din("evoT", [K, vhp], F16)        # stage-2 other-half stream
    decay = din("decay", [K, C], F32)
    a_re = din("a_re", [C, C], F16)
    a_imn = din("a_imn", [C, C], F16)   # -A_im
    w0a = din("w0a", [C, H], F16)
    w0b = din("w0b", [C, H], F16)
    w0c = din("w0c", [C, H], F16)
    w1 = din("w1", [H, C], F16)
    b0 = din("b0", [H, 1], F32)
    b1 = din("b1", [C, 1], F32)
    cols = din("cols", [nblk, 128, cap], I32)
    # rvxy packs rows | vx | vy along the last axis
    rvxy = din("rvxy", [nblk, 128, 3 * cap], F16)
    # sel[s] = 1.0 iff this core's sample is slot s (same program, data-driven)
    sel = din("sel", [128, 4], F32)

    outT = nc.dram_tensor("outT", [C, vhp], F32, kind="ExternalOutput").ap()
    if debug_taps:
        dbg_xs = nc.dram_tensor("dbg_xs", [K, C], F32, kind="ExternalOutput").ap()
        dbg_tab = nc.dram_tensor("dbg_tab", [1024, C], F16, kind="ExternalOutput").ap()
        dbg_r1 = nc.dram_tensor("dbg_r1", [C, 512], F16, kind="ExternalOutput").ap()
        dbg_xgt = nc.dram_tensor("dbg_xgt", [C, 256], F16, kind="ExternalOutput").ap()
        dbg_hr = nc.dram_tensor("dbg_hr", [H, 256], F16, kind="ExternalOutput").ap()
        dbg_g = nc.dram_tensor("dbg_g", [128, 2 * cap * C], F16, kind="ExternalOutput").ap()
        dbg_sp = nc.dram_tensor("dbg_sp", [128, cap * 256], F16, kind="ExternalOutput").ap()

    nslot = 4
    xs_loc = nc.dram_tensor("xs_loc", [nslot * K, C], F32).ap()
    xs_sh = nc.dram_tensor("xs_sh", [nslot * K, C], F32,
                           addr_space="Shared").ap()
    table = nc.dram_tensor("table", [tvp, C], F16).ap()

    groups = [list(range(n_cores))]

    with tile.TileContext(nc) as tc, ExitStack() as ctx:
        cpool = ctx.enter_context(tc.tile_pool(name="const", bufs=1))
        rpool = ctx.enter_context(tc.tile_pool(name="resid", bufs=1))

        # iota over r, twice side by side: [128, 2, 128] (for [Sx | Sy])
        iota_i = cpool.tile([128, 2, 128], I16, tag="iota_i")
        nc.gpsimd.iota(iota_i[:], pattern=[[0, 2], [1, 128]], base=0,
                       channel_multiplier=0)
        iota2 = cpool.tile([128, 2, 128], F16, tag="iota2")
        nc.vector.tensor_copy(out=iota2[:], in_=iota_i[:])

        # constants
        decay_sb = cpool.tile([K, C], F32, tag="decay")
        nc.sync.dma_start(out=decay_sb[:], in_=decay[:, :])
        are_sb = cpool.tile([C, C], F16, tag="are")
        nc.sync.dma_start(out=are_sb[:], in_=a_re[:, :])
        aimn_sb = cpool.tile([C, C], F16, tag="aimn")
        nc.sync.dma_start(out=aimn_sb[:], in_=a_imn[:, :])
        w0a_sb = cpool.tile([C, H], F16, tag="w0a")
        nc.sync.dma_start(out=w0a_sb[:], in_=w0a[:, :])
        w0b_sb = cpool.tile([C, H], F16, tag="w0b")
        nc.sync.dma_start(out=w0b_sb[:], in_=w0b[:, :])
        w0c_sb = cpool.tile([C, H], F16, tag="w0c")
        nc.sync.dma_start(out=w0c_sb[:], in_=w0c[:, :])
        w1_sb = cpool.tile([H, C], F16, tag="w1")
        nc.sync.dma_start(out=w1_sb[:], in_=w1[:, :])
        b0_sb = cpool.tile([H, 1], F32, tag="b0")
        nc.sync.dma_start(out=b0_sb[:], in_=b0[:, :])
        b1_sb = cpool.tile([C, 1], F32, tag="b1")
        nc.sync.dma_start(out=b1_sb[:], in_=b1[:, :])
        sel_sb = cpool.tile([128, 4], F32, tag="sel")
        nc.sync.dma_start(out=sel_sb[:], in_=sel[:, :])
        mass_sb = cpool.tile([128, nblk], F32, tag="mass")
        nc.sync.dma_start(out=mass_sb[:],
                          in_=mass[:, :].rearrange("(t p) one -> p (t one)",
                                                   p=128))

        # residents
        evT = rpool.tile([K, vhp], F16, tag="evT")
        nc.sync.dma_start(out=evT[:], in_=evT_own[:, :])
        xdT = rpool.tile([C, vhp], F16, tag="xdT")

        # ---------------- stage 1: spectral projection ----------------
        with tc.tile_pool(name="s1", bufs=3) as s1p, \
             tc.tile_pool(name="ps_xs", bufs=1, space="PSUM") as ps_xs:
            psum_xs = ps_xs.tile([K, C], F32, tag="xs")
            for q in range(nq):
                qsl = slice(q * 512, (q + 1) * 512)
                xin_q = s1p.tile([128, 4, C], F16, tag="xin")
                ev_q = s1p.tile([128, 4, K], F16, tag="ev")
                nc.sync.dma_start(
                    out=xin_q[:],
                    in_=x_in[qsl, :].rearrange("(a p) c -> p a c", p=128))
                nc.sync.dma_start(
                    out=ev_q[:],
                    in_=ev_own[qsl, :].rearrange("(a p) c -> p a c", p=128))
                mx_q = s1p.tile([128, 4, C], F16, tag="mx")
                for a in range(4):
                    t = 4 * q + a
                    nc.vector.tensor_scalar_mul(mx_q[:, a, :], xin_q[:, a, :],
                                                mass_sb[:, t:t + 1])
                    nc.tensor.matmul(psum_xs[:], lhsT=ev_q[:, a, :],
                                     rhs=mx_q[:, a, :],
                                     start=(t == 0), stop=(t == nblk - 1))

            xs_sb = cpool.tile([K, C], F32, tag="xs_sb")
            nc.scalar.activation(out=xs_sb[:], in_=psum_xs[:], func=AF.Copy)
            for s in range(nslot):
                slot_sb = s1p.tile([K, C], F32, tag="slot")
                nc.vector.tensor_scalar_mul(slot_sb[:], xs_sb[:],
                                            sel_sb[:, s:s + 1])
                nc.sync.dma_start(out=xs_loc[s * K:(s + 1) * K, :],
                                  in_=slot_sb[:])

        nc.gpsimd.collective_compute(
            "AllReduce", ALU.add,
            ins=[xs_loc[:, :]], outs=[xs_sh[:, :]],
            replica_groups=groups,
        )

        # xs = decay * (own slot of the AllReduce result)
        slots_sb = cpool.tile([K, nslot * C], F32, tag="slots")
        for s in range(nslot):
            nc.sync.dma_start(out=slots_sb[:, s * C:(s + 1) * C],
                              in_=xs_sh[s * K:(s + 1) * K, :])
        xs_ar = cpool.tile([K, C], F32, tag="xs_ar")
        nc.vector.tensor_scalar_mul(xs_ar[:], slots_sb[:, 0:C],
                                    sel_sb[:, 0:1])
        for s in range(1, nslot):
            nc.vector.scalar_tensor_tensor(
                out=xs_ar[:], in0=slots_sb[:, s * C:(s + 1) * C],
                scalar=sel_sb[:, s:s + 1], in1=xs_ar[:],
                op0=ALU.mult, op1=ALU.add)
        xs32 = cpool.tile([K, C], F32, tag="xs32")
        nc.vector.tensor_tensor(out=xs32[:], in0=xs_ar[:], in1=decay_sb[:],
                                op=ALU.mult)
        xs16 = cpool.tile([K, C], F16, tag="xs16")
        nc.vector.tensor_copy(out=xs16[:], in_=xs32[:])
        if debug_taps:
            nc.sync.dma_start(out=dbg_xs[:, :], in_=xs32[:])

        # ---------------- stage 2: x_diffuse table + xdT ----------------
        with tc.tile_pool(name="s2", bufs=3) as s2p, \
             tc.tile_pool(name="ps_xd", bufs=3, space="PSUM") as ps_xd, \
             tc.tile_pool(name="ps_xt", bufs=3, space="PSUM") as ps_xt:
            for q in range(nq):
                td_q = s2p.tile([128, 4, C], F16, tag="td")
                for a in range(4):
                    t = 4 * q + a
                    vsl = slice(t * 128, (t + 1) * 128)
                    pxd = ps_xd.tile([128, C], F32, tag="xd")
                    nc.tensor.matmul(pxd[:], lhsT=evT[:, vsl], rhs=xs16[:],
                                     start=True, stop=True)
                    nc.scalar.activation(out=td_q[:, a, :], in_=pxd[:],
                                         func=AF.Copy)
                    pxt = ps_xt.tile([C, 128], F32, tag="xdt")
                    nc.tensor.matmul(pxt[:], lhsT=xs16[:], rhs=evT[:, vsl],
                                     start=True, stop=True)
                    nc.vector.tensor_copy(out=xdT[:, vsl], in_=pxt[:])
                nc.sync.dma_start(
                    out=table[q * 512:(q + 1) * 512, :].rearrange(
                        "(a p) c -> p a c", p=128),
                    in_=td_q[:])
                if debug_taps and q == 0:
                    nc.sync.dma_start(
                        out=dbg_tab[0:512, :].rearrange("(a p) c -> p a c", p=128),
                        in_=td_q[:])

            for q in range(nq):
                evo_q = s2p.tile([128, 4, 128], F16, tag="evo")
                nc.sync.dma_start(
                    out=evo_q[:],
                    in_=evoT[:, q * 512:(q + 1) * 512].rearrange(
                        "p (a c) -> p a c", a=4))
                td_q = s2p.tile([128, 4, C], F16, tag="td")
                for a in range(4):
                    pxd = ps_xd.tile([128, C], F32, tag="xd")
                    nc.tensor.matmul(pxd[:], lhsT=evo_q[:, a, :], rhs=xs16[:],
                                     start=True, stop=True)
                    nc.scalar.activation(out=td_q[:, a, :], in_=pxd[:],
                                         func=AF.Copy)
                nc.sync.dma_start(
                    out=table[vhp + q * 512:vhp + (q + 1) * 512, :].rearrange(
                        "(a p) c -> p a c", p=128),
                    in_=td_q[:])

        # ---------------- stages 3-5: SpMM + rotation + MLP ----------------
        with tc.tile_pool(name="s4", bufs=3) as s4p, \
             tc.tile_pool(name="s4b", bufs=3) as s4bp, \
             tc.tile_pool(name="gch", bufs=3 * cap) as gchp, \
             tc.tile_pool(name="ps_gxy", bufs=2, space="PSUM") as ps_gxy, \
             tc.tile_pool(name="ps_vb", bufs=2, space="PSUM") as ps_vb, \
             tc.tile_pool(name="ps_h", bufs=2, space="PSUM") as ps_h, \
             tc.tile_pool(name="ps_o", bufs=2, space="PSUM") as ps_o:
            for sp2 in range(nsup // 2):
                # batched loads + per-chunk gathers for 2 superblocks (4 blks)
                cols_t = s4p.tile([128, 4, cap], I32, tag="cols")
                nc.sync.dma_start(
                    out=cols_t[:],
                    in_=cols[4 * sp2:4 * sp2 + 4].rearrange("a p j -> p a j"))
                meta_t = s4p.tile([128, 4, 3 * cap], F16, tag="meta")
                nc.sync.dma_start(
                    out=meta_t[:],
                    in_=rvxy[4 * sp2:4 * sp2 + 4].rearrange("a p j -> p a j"))
                # one indirect DMA per 128-edge chunk: the DGE consumes ONE
                # index per partition ([128,1] offsets -> [128,C] rows).
                # Separate per-chunk tiles (deep pool) so successive gathers
                # pipeline instead of WAW-serializing on one big tile, and so
                # each consuming matmul waits only for its own chunk.
                g_ch = [[None] * cap for _ in range(4)]
                for a4 in range(4):
                    for j4 in range(cap):
                        gt = gchp.tile([128, C], F16, tag="gch")
                        nc.gpsimd.indirect_dma_start(
                            out=gt[:], out_offset=None,
                            in_=table[:, :],
                            in_offset=bass.IndirectOffsetOnAxis(
                                ap=cols_t[:, a4, j4:j4 + 1], axis=0),
                        )
                        g_ch[a4][j4] = gt
                xinT_t = s4p.tile([C, 512], F16, tag="xinT")
                nc.sync.dma_start(out=xinT_t[:],
                                  in_=x_inT[:, sp2 * 512:(sp2 + 1) * 512])
                oT_t = s4p.tile([C, 512], F32, tag="oT")

                if debug_taps and sp2 == 0:
                    for a4 in range(2):
                        for j4 in range(cap):
                            nc.sync.dma_start(
                                out=dbg_g[:, (a4 * cap + j4) * C:
                                          (a4 * cap + j4 + 1) * C],
                                in_=g_ch[a4][j4][:])
                for si in range(2):
                    s = 2 * sp2 + si
                    ssl = slice(s * 256, (s + 1) * 256)
                    r1 = s4p.tile([C, 512], F16, tag="r1")
                    r2 = s4p.tile([C, 512], F16, tag="r2")
                    for i in range(2):
                        blk = 2 * si + i
                        # S = [eq*vx | eq*vy] built per chunk in one fused
                        # op: (iota2 == rows[p]) * [vx[p] | vy[p]]
                        sp_t = s4bp.tile([128, cap, 256], F16, tag="sp")
                        mv = meta_t[:].rearrange(
                            "p a (three j) -> p a three j", three=3)
                        for j in range(cap):
                            nc.vector.scalar_tensor_tensor(
                                out=sp_t[:, j, :].rearrange(
                                    "p (two r) -> p two r", two=2),
                                in0=iota2[:],
                                scalar=meta_t[:, blk, j:j + 1],
                                in1=mv[:, blk, 1:3, j].to_broadcast(
                                    [128, 2, 128]),
                                op0=ALU.is_equal, op1=ALU.mult)
                        if debug_taps and s == 0 and i == 0:
                            nc.sync.dma_start(out=dbg_sp[:, :],
                                              in_=sp_t[:].rearrange("p j r -> p (j r)"))
                        pgxy = ps_gxy.tile([C, 256], F32, tag="gxy")
                        for j in range(cap):
                            nc.tensor.matmul(
                                pgxy[:], lhsT=g_ch[blk][j][:],
                                rhs=sp_t[:, j, :],
                                start=(j == 0), stop=(j == cap - 1))
                        # r1 = [gxT | gyT], r2 = [gyT | -gxT] per block
                        nc.scalar.activation(out=r1[:, i * 256:(i + 1) * 256],
                                             in_=pgxy[:], func=AF.Copy)
                        nc.scalar.activation(out=r2[:, i * 256:i * 256 + 128],
                                             in_=pgxy[:, 128:256],
                                             func=AF.Copy)
                        nc.scalar.activation(
                            out=r2[:, i * 256 + 128:(i + 1) * 256],
                            in_=pgxy[:, 0:128], func=AF.Copy, scale=-1.0)

                    if debug_taps and s == 0:
                        nc.sync.dma_start(out=dbg_r1[:, :], in_=r1[:])
                    pvb = ps_vb.tile([C, 512], F32, tag="vb")
                    nc.tensor.matmul(pvb[:], lhsT=are_sb[:], rhs=r1[:],
                                     start=True, stop=False)
                    nc.tensor.matmul(pvb[:], lhsT=aimn_sb[:], rhs=r2[:],
                                     start=False, stop=True)

                    # x_grad^T = tanh(gx*vbre + gy*vbim)
                    pp = s4p.tile([C, 512], F16, tag="pp")
                    nc.vector.tensor_tensor(out=pp[:], in0=r1[:], in1=pvb[:],
                                            op=ALU.mult)
                    ppv = pp[:].rearrange("c (b two r) -> c b two r",
                                          two=2, r=128)
                    # keep this off gpsimd: the Pool queue also issues every
                    # indirect gather, so compute there stalls the gathers
                    xg = s4p.tile([C, 2, 128], F16, tag="xg")
                    nc.vector.tensor_tensor(out=xg[:], in0=ppv[:, :, 0, :],
                                            in1=ppv[:, :, 1, :], op=ALU.add)
                    xgt = s4p.tile([C, 256], F16, tag="xgt")
                    nc.scalar.activation(
                        out=xgt[:], in_=xg[:].rearrange("c b r -> c (b r)"),
                        func=AF.Tanh)
                    if debug_taps and s == 0:
                        nc.sync.dma_start(out=dbg_xgt[:, :], in_=xgt[:])

                    # MLP
                    xsl = slice(si * 256, (si + 1) * 256)
                    ph = ps_h.tile([H, 256], F32, tag="h")
                    nc.tensor.matmul(ph[:], lhsT=w0a_sb[:], rhs=xinT_t[:, xsl],
                                     start=True, stop=False)
                    nc.tensor.matmul(ph[:], lhsT=w0b_sb[:], rhs=xdT[:, ssl],
                                     start=False, stop=False)
                    nc.tensor.matmul(ph[:], lhsT=w0c_sb[:], rhs=xgt[:],
                                     start=False, stop=True)
                    hr = s4p.tile([H, 256], F16, tag="hr")
                    nc.scalar.activation(out=hr[:], in_=ph[:], func=AF.Relu,
                                         bias=b0_sb[:, :1])
                    if debug_taps and s == 0:
                        nc.sync.dma_start(out=dbg_hr[:, :], in_=hr[:])
                    po = ps_o.tile([C, 256], F32, tag="o")
                    nc.tensor.matmul(po[:], lhsT=w1_sb[:], rhs=hr[:],
                                     start=True, stop=True)
                    o1 = s4p.tile([C, 256], F32, tag="o1")
                    nc.scalar.activation(out=o1[:], in_=po[:],
                                         func=AF.Identity, bias=b1_sb[:, :1])
                    nc.vector.tensor_tensor(out=oT_t[:, xsl], in0=o1[:],
                                            in1=xinT_t[:, xsl], op=ALU.add)
                nc.sync.dma_start(out=outT[:, sp2 * 512:(sp2 + 1) * 512],
                                  in_=oT_t[:])

    _spill_waits(nc)
    return nc


# ---------------------------------------------------------------------------
# Host-side preprocessing
# ---------------------------------------------------------------------------
def host_prep(x_in, mass, L, evals, evecs, grad_rows, grad_cols,
              gradX_vals, gradY_vals, diffusion_time, A_re, A_im,
              W0, b0, W1, b1, cap=CAP_DEFAULT):
    """Build the 8 per-core input dicts. Returns (in_maps, cap_used)."""
    x_in = np.asarray(x_in, np.float32)
    mass = np.asarray(mass, np.float32)
    evals = np.asarray(evals, np.float32)
    evecs = np.asarray(evecs, np.float32)
    grad_rows = np.asarray(grad_rows)
    grad_cols = np.asarray(grad_cols)
    gradX_vals = np.asarray(gradX_vals, np.float32)
    gradY_vals = np.asarray(gradY_vals, np.float32)

    t = np.clip(np.asarray(diffusion_time, np.float32), 1e-8, None)
    W0 = np.asarray(W0, np.float32)
    W1 = np.asarray(W1, np.float32)
    b0 = np.asarray(b0, np.float32)
    b1 = np.asarray(b1, np.float32)
    A_re = np.asarray(A_re, np.float32)
    A_im = np.asarray(A_im, np.float32)

    need_cap = cap
    metas = []
    for b in range(B):
        r = grad_rows[b]
        for half in range(2):
            lo, hi = half * VH, (half + 1) * VH
            sel_e = (r >= lo) & (r < hi)
            cnt = np.bincount((r[sel_e] - lo) // 128, minlength=NBLK)
            need_cap = max(need_cap, int(np.ceil(cnt.max() / 128)))
            metas.append((b, half, sel_e))
    cap = int(need_cap)

    in_maps = []
    for b, half, sel_e in metas:
        lo = half * VH
        r = grad_rows[b][sel_e] - lo
        c = grad_cols[b][sel_e]
        vx = gradX_vals[b][sel_e]
        vy = gradY_vals[b][sel_e]
        order = np.argsort(r, kind="stable")
        r, c, vx, vy = r[order], c[order], vx[order], vy[order]

        # table row for a global col in this core's [own | other] table
        if half == 0:
            tc_ = np.where(c < VH, c, c + (VHP - VH))
        else:
            tc_ = np.where(c >= VH, c - VH, c + VHP)

        cols_a = np.zeros((NBLK, 128, cap), np.int32)
        rvxy_a = np.zeros((NBLK, 128, 3 * cap), np.float16)
        rvxy_a[:, :, 0:cap] = -1.0
        blk_of = r // 128
        starts = np.searchsorted(blk_of, np.arange(NBLK + 1))
        for blk in range(NBLK):
            s0, s1 = starts[blk], starts[blk + 1]
            n = s1 - s0
            assert n <= cap * 128
            j = np.arange(n) // 128
            p = np.arange(n) % 128
            cols_a[blk, p, j] = tc_[s0:s1]
            rvxy_a[blk, p, j] = (r[s0:s1] - blk * 128).astype(np.float16)
            rvxy_a[blk, p, cap + j] = vx[s0:s1].astype(np.float16)
            rvxy_a[blk, p, 2 * cap + j] = vy[s0:s1].astype(np.float16)

        xpad = np.zeros((VHP, C), np.float16)
        xpad[:VH] = x_in[b, lo:lo + VH]
        mpad = np.zeros((VHP, 1), np.float32)
        mpad[:VH, 0] = mass[b, lo:lo + VH]
        epad = np.zeros((VHP, K), np.float16)
        epad[:VH] = evecs[b, lo:lo + VH]
        oth = (1 - half) * VH
        evoT_a = np.zeros((K, VHP), np.float16)
        evoT_a[:, :VH] = evecs[b, oth:oth + VH].T
        evT_a = np.zeros((K, VHP), np.float16)
        evT_a[:, :VH] = evecs[b, lo:lo + VH].T
        x_inT_a = np.zeros((C, VHP), np.float16)
        x_inT_a[:, :VH] = x_in[b, lo:lo + VH].T

        decay = np.exp(-evals[b][:, None] * t[None, :]).astype(np.float32)

        in_maps.append({
            "x_in": xpad, "x_inT": x_inT_a, "mass": mpad,
            "ev_own": epad, "evT_own": evT_a, "evoT": evoT_a,
            "decay": decay,
            "a_re": A_re.astype(np.float16),
            "a_imn": (-A_im).astype(np.float16),
            "w0a": W0[0:C].astype(np.float16),
            "w0b": W0[C:2 * C].astype(np.float16),
            "w0c": W0[2 * C:3 * C].astype(np.float16),
            "w1": W1.astype(np.float16),
            "b0": b0.reshape(H, 1).astype(np.float32),
            "b1": b1.reshape(C, 1).astype(np.float32),
            "cols": cols_a, "rvxy": rvxy_a,
            "sel": np.repeat(np.eye(4, dtype=np.float32)[b][None, :], 128,
                             axis=0),
        })
    return in_maps, cap


_NC_CACHE = {}


def _get_nc(cap):
    if cap not in _NC_CACHE:
        _NC_CACHE[cap] = build_nc(cap=cap)
    return _NC_CACHE[cap]


def assemble(res) -> np.ndarray:
    out = np.empty((B, V, C), np.float32)
    for i in range(N_CORES):
        b, half = i // 2, i % 2
        out[b, half * VH:(half + 1) * VH] = res.results[i]["outT"].T[:VH]
    return out


def kernel(**inputs) -> np.ndarray:
    in_maps, cap = host_prep(**inputs)
    nc = _get_nc(cap)
    res = run_bass_kernel_spmd(nc, in_maps, list(range(N_CORES)))
    return assemble(res)

